# revision 15
# baseline (speedup 1.0000x reference)
"""Causal single-head attention (B=4, S=2048, D=768) on 8 TRN2 NeuronCores.

Sharding: core (b, h) = batch b, sequence-half h. Each core computes the
attention output for 1024 query rows (h=0: rows 0:512 + 1024:1536, h=1:
rows 512:1024 + 1536:2048) of one batch.

v4: keys stay in ORIGINAL order (no rotation). The K/V projections are
split across the batch-pair — each core projects only its own 1024 keys,
halves exchanged via 2-core HBM AllGathers (K in two et-chunks so the
first gather launches early; readbacks ride the gpsimd queue). Causality
and the per-core dead key tiles are applied with one multiplicative 0/1
mask-template input (per-core content, compile-time slide offsets), so
the SPMD program is identical on every core. All scores run before all
PV accumulations so the V gather hides behind score matmuls. bf16.
"""

import os
import numpy as np
import ml_dtypes

import concourse.bass as bass
import concourse.mybir as mybir
import concourse.tile as tile
from concourse import bacc
from concourse.bass_utils import run_bass_kernel_spmd

B, S, D = 4, 2048, 768
H = S // 2           # query rows / keys-owned per core
P = 128
ND = D // P          # 6  d/e tiles
NKH = H // P         # 8  key tiles owned per core
NK = S // P          # 16 key tiles total
SCALE = 1.0 / float(np.sqrt(D))
F32 = mybir.dt.float32
BF16 = mybir.dt.bfloat16
F8 = mybir.dt.float8e4
NPBF16 = np.dtype(ml_dtypes.bfloat16)
MW = 1408            # mask template width
MC = 896             # mask slice-offset constant
NEA = 3              # et tiles in first K gather chunk

_cached = {}
last_results = None


def _build_nc():
    nc = bacc.Bacc("TRN2", target_bir_lowering=False)

    xq_d = nc.dram_tensor("xq", [2 * D, 512], BF16, kind="ExternalInput")
    xkv_d = nc.dram_tensor("xkv", [2 * D, 512], BF16, kind="ExternalInput")
    wqT_d = nc.dram_tensor("wqT", [D, D], BF16, kind="ExternalInput")
    wkT_d = nc.dram_tensor("wkT", [D, D], BF16, kind="ExternalInput")
    wvT_d = nc.dram_tensor("wvT", [D, D], BF16, kind="ExternalInput")
    pm_d = nc.dram_tensor("pmask", [P, MW], BF16, kind="ExternalInput")
    out_d = nc.dram_tensor("out", [H, D], BF16, kind="ExternalOutput")

    groups = [[0, 1], [2, 3], [4, 5], [6, 7]]

    with tile.TileContext(nc) as tc:
        with (
            tc.tile_pool(name="qtp", bufs=ND) as qtp,
            tc.tile_pool(name="ktp", bufs=ND) as ktp,
            tc.tile_pool(name="vp", bufs=NKH + 2) as vp,
            tc.tile_pool(name="vgp", bufs=1) as vgp,
            tc.tile_pool(name="cst", bufs=1) as cst,
            tc.tile_pool(name="dram", bufs=6, space="DRAM") as dram,
        ):
            pm = cst.tile([P, MW], BF16)
            nc.sync.dma_start(out=pm[:], in_=pm_d[:, :])

            k_inA = dram.tile([NEA * P, H], BF16)
            k_inB = dram.tile([(ND - NEA) * P, H], BF16)
            k_outA = dram.tile([2 * NEA * P, H], BF16)
            k_outB = dram.tile([2 * (ND - NEA) * P, H], BF16)
            v_in = dram.tile([NKH * P, D + 2], BF16)
            v_out = dram.tile([2 * NKH * P, D + 2], BF16)

            qts, kts, vs = [], [], []
            # ---- projections over own data ----
            with (
                tc.tile_pool(name="xp", bufs=2 * ND) as xp,
                tc.tile_pool(name="khp", bufs=ND) as khp,
                tc.tile_pool(name="wp", bufs=3 * ND) as wp,
                tc.tile_pool(name="psj", bufs=8, space="PSUM") as psj,
            ):
                # all input DMAs issued up-front (sync queue drains in
                # arrival order — no head-of-line blocking on staging):
                # wk et0-slices + first xkv chunk feed the first matmuls.
                xkv = [xp.tile([P, H], BF16, name=f"xkv{d}", tag="xt") for d in range(ND)]
                xq = [xp.tile([P, H], BF16, name=f"xq{d}", tag="xt") for d in range(ND)]
                wk, wv, wq = [], [], []
                for wn, w_list in (("wk", wk), ("wv", wv), ("wq", wq)):
                    for d in range(ND):
                        w_list.append(
                            wp.tile([P, D], BF16, name=f"{wn}{d}", tag="w"))
                def xblk(x_dram, d, c):
                    r = (2 * d + c) * P
                    return x_dram[r:r + P, :]

                for d in range(ND):
                    nc.sync.dma_start(out=wk[d][:], in_=wkT_d[d * P:(d + 1) * P, :])
                    nc.sync.dma_start(out=xkv[d][:, 0:512], in_=xblk(xkv_d, d, 0))
                for d in range(ND):
                    nc.sync.dma_start(out=xkv[d][:, 512:H], in_=xblk(xkv_d, d, 1))
                for d in range(ND):
                    nc.sync.dma_start(out=wv[d][:], in_=wvT_d[d * P:(d + 1) * P, :])
                for d in range(ND):
                    nc.sync.dma_start(out=wq[d][:], in_=wqT_d[d * P:(d + 1) * P, :])
                    nc.sync.dma_start(out=xq[d][:, 0:512], in_=xblk(xq_d, d, 0))
                    nc.sync.dma_start(out=xq[d][:, 512:H], in_=xblk(xq_d, d, 1))

                # own-half KT[e, 0:1024] = sum_d wkT[d,e]^T xkv[d, :]
                for et in range(ND):
                    kh = khp.tile([P, H], BF16)
                    for c0 in (0, 512):
                        acc = psj.tile([P, 512], F32, tag="ps")
                        for d in range(ND):
                            nc.tensor.matmul(
                                acc[:],
                                wk[d][:, et * P:(et + 1) * P],
                                xkv[d][:, c0:c0 + 512],
                                start=(d == 0), stop=(d == ND - 1),
                            )
                        nc.vector.tensor_copy(kh[:, c0:c0 + 512], acc[:])
                    if et < NEA:
                        nc.gpsimd.dma_start(out=k_inA[et * P:(et + 1) * P, :], in_=kh[:])
                    else:
                        et2 = et - NEA
                        nc.gpsimd.dma_start(out=k_inB[et2 * P:(et2 + 1) * P, :], in_=kh[:])
                    if et == NEA - 1:
                        nc.gpsimd.collective_compute(
                            "AllGather", mybir.AluOpType.bypass,
                            replica_groups=groups,
                            ins=[k_inA.opt()], outs=[k_outA.opt()],
                        )
                # gathered-K readback (gpsimd queue; waits the
                # collectives). A-chunk tiles ride during gather B.
                for et in range(ND):
                    kt = ktp.tile([P, S], BF16)
                    kts.append(kt)
                nc.gpsimd.collective_compute(
                    "AllGather", mybir.AluOpType.bypass,
                    replica_groups=groups,
                    ins=[k_inB.opt()], outs=[k_outB.opt()],
                )
                for et in range(NEA):
                    for rk in range(2):
                        r0 = (rk * NEA + et) * P
                        nc.gpsimd.dma_start(
                            out=kts[et][:, rk * H:(rk + 1) * H],
                            in_=k_outA[r0:r0 + P, :])

                # own-half V[j, e] for the core's 8 key tiles
                for jt in range(NKH):
                    v = vp.tile([P, D + 2], BF16, tag="vh")
                    for e0, ew in ((0, 512), (512, 256)):
                        acc = psj.tile([P, 512], F32, tag="ps")
                        for d in range(ND):
                            nc.tensor.matmul(
                                acc[:, :ew],
                                xkv[d][:, jt * P:(jt + 1) * P],
                                wv[d][:, e0:e0 + ew],
                                start=(d == 0), stop=(d == ND - 1),
                            )
                        nc.vector.tensor_copy(v[:, e0:e0 + ew], acc[:, :ew])
                    ones = nc.const_aps.tensor(1.0, (P, 2), BF16)
                    nc.vector.tensor_copy(v[:, D:D + 2], ones)
                    nc.gpsimd.dma_start(out=v_in[jt * P:(jt + 1) * P, :], in_=v[:])

                nc.gpsimd.collective_compute(
                    "AllGather", mybir.AluOpType.bypass,
                    replica_groups=groups,
                    ins=[v_in.opt()], outs=[v_out.opt()],
                )
                for et in range(NEA, ND):
                    for rk in range(2):
                        r0 = (rk * (ND - NEA) + et - NEA) * P
                        nc.gpsimd.dma_start(
                            out=kts[et][:, rk * H:(rk + 1) * H],
                            in_=k_outB[r0:r0 + P, :])
                vg = vgp.tile([P, NK * (D + 2)], BF16, tag="vg")
                vr = v_out.opt().rearrange("(jt p) j -> p jt j", p=P)
                for j4 in range(0, NK, 4):
                    nc.gpsimd.dma_start(
                        out=vg[:, j4 * (D + 2):(j4 + 4) * (D + 2)],
                        in_=vr[:, j4:j4 + 4, :])
                for jt in range(NK):
                    vs.append(vg[:, jt * (D + 2):(jt + 1) * (D + 2)])

                # QT[e, i] = sum_d wqT[d,e]^T xq[d, i]
                for et in range(ND):
                    qt = qtp.tile([P, H], BF16)
                    qts.append(qt)
                    for c0 in (0, 512):
                        acc = psj.tile([P, 512], F32, tag="ps")
                        for d in range(ND):
                            nc.tensor.matmul(
                                acc[:],
                                wq[d][:, et * P:(et + 1) * P],
                                xq[d][:, c0:c0 + 512],
                                start=(d == 0), stop=(d == ND - 1),
                            )
                        nc.vector.tensor_copy(qt[:, c0:c0 + 512], acc[:])

            # ---- attention: transposed scores over 512-query quads ----
            # scoresT[j, i] tiles [128, 512]. pt = exp(scale*st) * mask,
            # mask = pm[:, s:s+512], s = MC - 128*jt + 1024*q — a 0/1
            # sliding template whose content encodes the core's h offset.
            # All 24 score tiles first (so the V gather hides behind
            # them), then the 8 PV sweeps over the per-half live sets;
            # ones columns of V give the softmax denominator.
            with (
                tc.tile_pool(name="ptp", bufs=24) as ptp,
                tc.tile_pool(name="pt2p", bufs=4) as pt2p,
                tc.tile_pool(name="sgp", bufs=4) as sgp,
                tc.tile_pool(name="op", bufs=3) as op,
                tc.tile_pool(name="ps", bufs=4, space="PSUM") as ps_pool,
                tc.tile_pool(name="pspv", bufs=2, space="PSUM") as pspv_pool,
            ):
                pts = {}
                tiles = [(0, jt) for jt in range(8)] + [(1, jt) for jt in range(NK)]
                # phase 1: partial scores over the et<NEA K tiles (from
                # gather A) — exp'd and masked while gather B transfers.
                for q, jt in tiles:
                    qc = 512 * q
                    st = ps_pool.tile([P, 512], F32, tag="st")
                    for et in range(NEA):
                        nc.tensor.matmul(
                            st[:],
                            kts[et][:, jt * P:(jt + 1) * P],
                            qts[et][:, qc:qc + 512],
                            start=(et == 0), stop=(et == NEA - 1),
                        )
                    pt = ptp.tile([P, 512], BF16, tag="pt")
                    pts[q, jt] = pt
                    nc.scalar.activation(
                        pt[:], st[:], mybir.ActivationFunctionType.Exp,
                        bias=0.0, scale=SCALE,
                    )
                    if q == 0 or jt >= 8:
                        s = MC - P * jt + 1024 * q
                        nc.vector.scalar_tensor_tensor(
                            out=pt[:], in0=pt[:], scalar=1.0,
                            in1=pm[:, s:s + 512],
                            op0=mybir.AluOpType.mult,
                            op1=mybir.AluOpType.mult,
                        )
                # phase 2: the et>=NEA remainder; exp(a+b) = exp(a)*exp(b)
                for q, jt in tiles:
                    qc = 512 * q
                    st = ps_pool.tile([P, 512], F32, tag="st")
                    for et in range(NEA, ND):
                        nc.tensor.matmul(
                            st[:],
                            kts[et][:, jt * P:(jt + 1) * P],
                            qts[et][:, qc:qc + 512],
                            start=(et == NEA), stop=(et == ND - 1),
                        )
                    pt2 = pt2p.tile([P, 512], BF16, tag="pt2")
                    nc.scalar.activation(
                        pt2[:], st[:], mybir.ActivationFunctionType.Exp,
                        bias=0.0, scale=SCALE,
                    )
                    nc.vector.scalar_tensor_tensor(
                        out=pts[q, jt][:], in0=pts[q, jt][:], scalar=1.0,
                        in1=pt2[:],
                        op0=mybir.AluOpType.mult,
                        op1=mybir.AluOpType.mult,
                    )
                for q, half in [(0, 0), (0, 1), (0, 2), (0, 3),
                                (1, 3), (1, 2), (1, 1), (1, 0)]:
                    qc = 512 * q
                    jts_half = list(range(5 + half + 8 * q))
                    pv = pspv_pool.tile([P, D + 2], F32, tag="pv")
                    h0 = half * P
                    for idx, jt in enumerate(jts_half):
                        for e0, ew in ((0, 512), (512, D + 2 - 512)):
                            nc.tensor.matmul(
                                pv[:, e0:e0 + ew],
                                pts[q, jt][:, h0:h0 + P],
                                vs[jt][:, e0:e0 + ew],
                                start=(idx == 0), stop=(idx == len(jts_half) - 1),
                            )
                    rcp = sgp.tile([P, 1], F32, tag="rcp")
                    nc.vector.reciprocal(rcp[:], pv[:, D:D + 1])
                    o = op.tile([P, D], BF16, tag="o")
                    nc.vector.tensor_scalar_mul(o[:], pv[:, :D], rcp[:])
                    r0 = qc + h0
                    nc.sync.dma_start(out=out_d[r0:r0 + P, :], in_=o[:])

    nc.compile()
    return nc


def _get_nc():
    if "nc" not in _cached:
        _cached["nc"] = _build_nc()
    return _cached["nc"]


def kernel(x, w_q, w_k, w_v):
    global last_results
    x = np.asarray(x, dtype=np.float32)
    wqT = np.ascontiguousarray(np.asarray(w_q, dtype=np.float32).T.astype(NPBF16))
    wkT = np.ascontiguousarray(np.asarray(w_k, dtype=np.float32).T.astype(NPBF16))
    wvT = np.ascontiguousarray(np.asarray(w_v, dtype=np.float32).T.astype(NPBF16))

    y = np.arange(MW)[None, :]
    p = np.arange(P)[:, None]

    nc = _get_nc()
    in_maps = []
    for core in range(8):
        b, h = core // 2, core % 2
        r = 512 * h
        xb = x[b]
        xq = np.concatenate([xb[r:r + 512], xb[1024 + r:1536 + r]], axis=0)
        xkv = xb[1024 * h:1024 * h + H]
        # pack [D, 1024] -> [d-tile][col-chunk][128, 512] contiguous blocks
        def pack(a2):
            return np.ascontiguousarray(
                a2.T.astype(NPBF16).reshape(ND, P, 2, 512)
                .transpose(0, 2, 1, 3).reshape(2 * D, 512))
        pmask = (y >= p + MC - 512 * h).astype(NPBF16)
        in_maps.append({
            "xq": pack(xq),
            "xkv": pack(xkv),
            "wqT": wqT, "wkT": wkT, "wvT": wvT,
            "pmask": pmask,
        })

    trace = bool(int(os.environ.get("KERNEL_TRACE", "0")))
    res = run_bass_kernel_spmd(nc, in_maps, core_ids=list(range(8)), trace=trace)
    last_results = res

    out = np.empty((B, S, D), np.float32)
    for core in range(8):
        b, h = core // 2, core % 2
        r = 512 * h
        o = np.asarray(res.results[core]["out"]).astype(np.float32)
        out[b, r:r + 512] = o[0:512]
        out[b, 1024 + r:1024 + r + 512] = o[512:1024]
    return out
